# revision 48
# baseline (speedup 1.0000x reference)
"""Trainium2 Bass kernel for batched GCN (2x GCNConv + circular Conv1d).

Math per graph (N=64 nodes, S=96 feats, H=512 hidden, E=512 edges):
    deg[d]   = #edges with dst=d (+1 self loop)
    Msb2     = Dinv A^T Dinv  (A[d,s] edge counts + I; Dinv = diag deg^-1/2)
    xa       = x @ Msb2                  # GCN1 aggregation first (reassoc)
    h1       = relu(W1 @ xa)             # [512, 64] (transposed layout)
    h2       = Msb2^T-agg(h1^T W2^T)     # [64, 96]
    y        = circular_conv1d(h2)       # [96, 512]

v2 design (cost-model driven):
  - Host pre-lays out x (node-major bf16) and edges (chunk-major bf16):
    no on-core transposes/casts for inputs.
  - One-hots via one DVE is_equal against an iota table with a broadcast
    input AP (no materialized broadcast copy).
  - A built block-diagonally in PSUM per pair (2 graphs); deg via 8
    free-size-1 matmuls; +1 self-loop via Sqrt bias.
  - Both dinv factors folded into the two PSUM->SBUF copies of A.
  - Conv with K-stacked taps: 2 matmuls per graph (taps{-1,0} K=128,
    tap{+1} K=64) over a halo'd shifted tile.
  - y stored bf16 (host upcasts), one DMA per pair, 1KB+ elems.
  - Elementwise spread across DVE/Act/Pool to sit under the PE bound.
"""

import numpy as np
import ml_dtypes

import concourse.bacc as bacc
import concourse.mybir as mybir
import concourse.tile as tile
from concourse.bass_utils import run_bass_kernel_spmd

BF16 = mybir.dt.bfloat16
FP32 = mybir.dt.float32
AF = mybir.ActivationFunctionType

N_CORES = 8
B, S, N, H, E = 512, 96, 64, 512, 512
G = B // N_CORES          # graphs per core
NPAIR = G // 2


def build_gcn_kernel(tc, outs, ins, g_per_core=G, has_b1=False, has_b2=False,
                     stage=6):
    nc = tc.nc
    g = g_per_core
    npair = g // 2

    xt_d = ins["xt"]        # [128, npair*96] bf16  (rows (gl,n), free (pr,s))
    et_d = ins["et"]        # [128, 512] bf16       (rows p, free (c, gt))
    w1t_d = ins["w1t"]      # [96, 512] bf16
    w2t_d = ins["w2t"]      # [128, 384] bf16  (rows h-in-chunk, free (c, s))
    cwk1_d = ins["cwk1"]    # [128, 512] bf16  (rows (k01, i), free o)
    cwk2_d = ins["cwk2"]    # [64, 512] bf16   (rows i (k=2), free o)
    iota_d = ins["iota"]    # [128, 1024] bf16 (f%64)
    i64d_d = ins["i64d"]    # [128, 64] bf16 (I64 stacked twice)
    id128_d = ins["id128"]  # [128, 128] bf16
    y_d = outs["y"]         # [g, 96, 512] bf16

    from contextlib import ExitStack
    ctx = ExitStack()
    const = ctx.enter_context(tc.tile_pool(name="const", bufs=1))
    sb = ctx.enter_context(tc.tile_pool(name="sb", bufs=3))
    sbh = ctx.enter_context(tc.tile_pool(name="sbh", bufs=2))
    pp = ctx.enter_context(tc.tile_pool(name="pp", bufs=2, space="PSUM"))
    pa = ctx.enter_context(tc.tile_pool(name="pa", bufs=2, space="PSUM"))
    pq = ctx.enter_context(tc.tile_pool(name="pq", bufs=2, space="PSUM"))
    py = ctx.enter_context(tc.tile_pool(name="py", bufs=2, space="PSUM"))

    # ---- constants ----
    xt = const.tile([128, npair * 96], BF16)
    nc.sync.dma_start(out=xt[:], in_=xt_d[:])
    et = const.tile([128, 512], BF16)
    nc.sync.dma_start(out=et[:], in_=et_d[:])
    w1t = const.tile([96, 512], BF16)
    nc.sync.dma_start(out=w1t[:], in_=w1t_d[:])
    w2t = const.tile([128, 384], BF16)
    nc.sync.dma_start(out=w2t[:], in_=w2t_d[:])
    cwk1 = const.tile([128, 512], BF16)
    nc.sync.dma_start(out=cwk1[:], in_=cwk1_d[:])
    cwk2 = const.tile([64, 512], BF16)
    nc.sync.dma_start(out=cwk2[:], in_=cwk2_d[:])
    iota = const.tile([128, 1024], BF16)
    nc.sync.dma_start(out=iota[:], in_=iota_d[:])
    i64d = const.tile([128, 64], BF16)
    nc.sync.dma_start(out=i64d[:], in_=i64d_d[:])
    # id_dup = [I[:, 0:64] | I[:, 0:64] | I[:, 64:128] | I[:, 64:128]] so the
    # transposing matmul emits each graph's Msb2 block twice, contiguously.
    iddup = const.tile([128, 256], BF16)
    nc.sync.dma_start(out=iddup[:], in_=id128_d[:])
    ones = const.tile([128, 1], BF16)
    nc.vector.memset(ones[:], 1.0)
    zeros64 = const.tile([128, 64], BF16)
    nc.vector.memset(zeros64[:], 0.0)
    if has_b1:
        b1c = const.tile([128, 4], FP32)
        nc.sync.dma_start(out=b1c[:], in_=ins["b1c"][:])
    if has_b2:
        b2w = const.tile([128, 98], BF16)
        nc.sync.dma_start(out=b2w[:], in_=ins["b2w"][:])

    et_c = et[:].rearrange("p (c gt) -> p c gt", c=4)

    for pr in range(npair):
        q = pr % 2          # position within quad
        if q == 0:
            a2x = pq.tile([128, 512], FP32, tag="a2x")
            a2q = a2x[:, 0:384]

        # ---- one-hots: oh[p, (c, j, v)] = (et[p, (c, 4pr+j)] == v) ----
        e_sl = et_c[:, :, 4 * pr:4 * pr + 4]
        e_sl = e_sl.rearrange("p c (j u) -> p c j u", u=1)
        e_bc = e_sl.to_broadcast([128, 4, 4, 64])
        erep = sb.tile([128, 1024], BF16, tag="erep")
        nc.gpsimd.tensor_copy(
            out=erep[:].rearrange("p (c j v) -> p c j v", c=4, j=4),
            in_=e_bc)
        oh = sb.tile([128, 1024], BF16, tag="oh")
        nc.vector.tensor_tensor(
            out=oh[:], in0=erep[:], in1=iota[:],
            op=mybir.AluOpType.is_equal,
        )

        # ---- pair psum tile: maug_bd | deg | mst(dup) | z2 ----
        mps = pp.tile([128, 496], FP32, tag="mps")
        maug = mps[:, 0:128]     # block-diag A+I, rows dst, cols src
        deg = mps[:, 128:129]
        mst = mps[:, 144:400]    # [B0|B0|B1|B1] after dup-transpose
        z2_ps = mps[:, 400:496]

        for gl in range(2):
            po = 64 * gl
            out_sl = maug[po:po + 64, po:po + 64]
            for c in range(4):
                base = c * 256
                lhsT = oh[:, base + (2 * gl + 1) * 64: base + (2 * gl + 2) * 64]
                rhs = oh[:, base + (2 * gl) * 64: base + (2 * gl + 1) * 64]
                nc.tensor.matmul(out_sl, lhsT, rhs, start=(c == 0), stop=False)
            nc.tensor.matmul(
                out_sl, i64d[po:po + 64, :], i64d[po:po + 64, :],
                start=False, stop=True)
            # zero the off-diag quadrant (rows gl, cols 1-gl)
            nc.tensor.matmul(
                maug[po:po + 64, 64 - po:128 - po],
                zeros64[:, :], oh[:, 0:64], start=True, stop=True)

        # ---- deg (incl self-loop) -> dinv = 1/sqrt(deg) ----
        nc.vector.tensor_reduce(out=deg, in_=maug,
                                axis=mybir.AxisListType.X,
                                op=mybir.AluOpType.add)
        sq = sb.tile([128, 1], FP32, tag="sq")
        nc.scalar.activation(out=sq[:], in_=deg, func=AF.Sqrt)
        dinv = sb.tile([128, 1], FP32, tag="dinv")
        nc.vector.reciprocal(out=dinv[:], in_=sq[:])

        # ---- msb = maug * dinv[dst] (block-diag, bf16) ----
        msb = sb.tile([128, 128], BF16, tag="msb")
        nc.vector.tensor_scalar(out=msb[:], in0=maug, scalar1=dinv[:, :],
                                scalar2=None, op0=mybir.AluOpType.mult)
        # ---- transpose-dup via matmul: mst = msb^T @ id_dup ----
        nc.tensor.matmul(mst, msb[:], iddup[:], start=True, stop=True)
        msbd = sb.tile([128, 256], BF16, tag="msbd")
        nc.scalar.activation(out=msbd[:], in_=mst, func=AF.Copy,
                             scale=dinv[:, :])

        if stage < 3:
            continue
        # ---- xa = x @ Msb2 : [96 (s), 128 (g,d)], one mm per col block ----
        xa_ps = a2x[0:96, 384:512]
        nc.tensor.matmul(xa_ps[:, 0:64], xt[:, 96 * pr:96 * (pr + 1)],
                         msbd[:, 0:64], start=True, stop=True)
        nc.tensor.matmul(xa_ps[:, 64:128], xt[:, 96 * pr:96 * (pr + 1)],
                         msbd[:, 192:256], start=True, stop=True)
        xab = sb.tile([96, 128], BF16, tag="xab")
        nc.vector.tensor_copy(out=xab[:], in_=xa_ps)

        # ---- a1t = W1 @ xa : [128 (h-chunk), (c, g, d)] ----
        a1t = pa.tile([128, 512], FP32, tag="a1t")
        for c in range(4):
            nc.tensor.matmul(a1t[:, 128 * c:128 * (c + 1)],
                             w1t[:, 128 * c:128 * (c + 1)], xab[:],
                             start=True, stop=True)
        h1t = sb.tile([128, 512], BF16, tag="h1t")
        if has_b1:
            for c in range(4):
                nc.scalar.activation(
                    out=h1t[:, 128 * c:128 * (c + 1)],
                    in_=a1t[:, 128 * c:128 * (c + 1)],
                    func=AF.Relu, bias=b1c[:, c:c + 1])
        else:
            nc.scalar.activation(out=h1t[:, 0:320], in_=a1t[:, 0:320],
                                 func=AF.Relu)
            nc.vector.tensor_scalar_max(h1t[:, 320:512], a1t[:, 320:512], 0.0)

        if stage < 4:
            continue
        # ---- z2 = h1 W2^T : [128 (g,n), 96 (s)] ----
        for c in range(4):
            nc.tensor.matmul(z2_ps, h1t[:, 128 * c:128 * (c + 1)],
                             w2t[:, 96 * c:96 * (c + 1)],
                             start=(c == 0), stop=(c == 3))
        z2s = sb.tile([128, 96], BF16, tag="z2s")
        nc.vector.tensor_copy(out=z2s[:], in_=z2_ps)

        # ---- a2 = Msb2^T-agg -> h2 duplicated into both row blocks ----
        # lhsT = [B_gl | B_gl] (contiguous in msbd thanks to the dup
        # transpose), so one M=128 matmul per graph writes rows 0:64 and
        # 64:128 identically.
        # lhsT's off-graph rows are zero, so contracting over the full 128
        # partitions selects graph gl alone — keeps all operands at base 0.
        for gl in range(2):
            w0 = 96 * (2 * q + gl)
            nc.tensor.matmul(a2q[0:128, w0:w0 + 96],
                             msbd[:, 128 * gl:128 * gl + 128], z2s[:, :],
                             start=True, stop=True)

        if q == 1 and stage < 5:
            for qq in range(4):
                ysb = sbh.tile([96, 512], BF16, tag="ysb")
                nc.vector.memset(ysb[:], 0.0)
                nc.sync.dma_start(out=y_d[4 * (pr // 2) + qq], in_=ysb[:])
        if q == 1 and stage >= 5:
            # ---- hpw: [128, (4, 98)]; block0 = h2[l-1] + halos, block1 = h2[l] ----
            hpw = sbh.tile([128, 392], BF16, tag="hpw")
            hpw0 = hpw[0:64, :].rearrange("p (qq w) -> p qq w", w=98)
            hpw1 = hpw[64:128, :].rearrange("p (qq w) -> p qq w", w=98)
            a2q0 = a2q[0:64, :].rearrange("p (qq w) -> p qq w", w=96)
            a2q1 = a2q[64:128, :].rearrange("p (qq w) -> p qq w", w=96)
            if has_b2:
                b2w0 = b2w[0:64, :].rearrange("p (qq w) -> p qq w", qq=1)
                b2w1 = b2w[64:128, :].rearrange("p (qq w) -> p qq w", qq=1)
                nc.vector.tensor_tensor(
                    out=hpw0[:, :, 1:97], in0=a2q0[:, :, 0:96],
                    in1=b2w0[:, :, 1:97].to_broadcast([64, 4, 96]),
                    op=mybir.AluOpType.add)
                nc.vector.tensor_tensor(
                    out=hpw1[:, :, 0:96], in0=a2q1[:, :, 0:96],
                    in1=b2w1[:, :, 0:96].to_broadcast([64, 4, 96]),
                    op=mybir.AluOpType.add)
                nc.vector.tensor_tensor(
                    out=hpw0[:, :, 0:1], in0=a2q0[:, :, 95:96],
                    in1=b2w0[:, :, 0:1].to_broadcast([64, 4, 1]),
                    op=mybir.AluOpType.add)
                nc.vector.tensor_tensor(
                    out=hpw0[:, :, 97:98], in0=a2q0[:, :, 0:1],
                    in1=b2w0[:, :, 97:98].to_broadcast([64, 4, 1]),
                    op=mybir.AluOpType.add)
            else:
                nc.scalar.activation(out=hpw0[:, :, 1:97],
                                     in_=a2q0[:, :, 0:96], func=AF.Copy)
                nc.vector.tensor_copy(out=hpw1[:, :, 0:96],
                                      in_=a2q1[:, :, 0:96])
                nc.vector.tensor_copy(out=hpw0[:, :, 0:1],
                                      in_=a2q0[:, :, 95:96])
                nc.scalar.activation(out=hpw0[:, :, 97:98],
                                     in_=a2q0[:, :, 0:1], func=AF.Copy)

            # ---- conv: per graph 2 K-stacked matmuls -> [96 (l), 512 (o)] ----
            for qq in range(4):
                if stage < 6:
                    ysb = sbh.tile([96, 512], BF16, tag="ysb")
                    nc.vector.memset(ysb[:], 0.0)
                    nc.sync.dma_start(out=y_d[4 * (pr // 2) + qq], in_=ysb[:])
                    continue
                y_ps = py.tile([96, 512], FP32, tag="y")
                nc.tensor.matmul(y_ps[:], hpw[:, 98 * qq:98 * qq + 96],
                                 cwk1[:], start=True, stop=False)
                nc.tensor.matmul(y_ps[:], hpw[0:64, 98 * qq + 2:98 * qq + 98],
                                 cwk2[:], start=False, stop=True)
                ysb = sbh.tile([96, 512], BF16, tag="ysb")
                if qq % 2 == 0:
                    nc.vector.tensor_copy(out=ysb[:], in_=y_ps[:])
                else:
                    nc.scalar.activation(out=ysb[:], in_=y_ps[:],
                                         func=AF.Copy)
                nc.sync.dma_start(out=y_d[4 * (pr // 2) + qq], in_=ysb[:])

    ctx.close()


# ---------------- host side ----------------

def _prep_consts(W1, b1, W2, b2, conv_w):
    bf = ml_dtypes.bfloat16
    w1t = np.ascontiguousarray(W1.T).astype(bf)                    # [96, 512]
    w2t = np.ascontiguousarray(
        W2.T.reshape(4, 128, 96).transpose(1, 0, 2).reshape(128, 384)
    ).astype(bf)
    cwt = conv_w.transpose(2, 1, 0)                                # [3, 64, 512]
    cwk1 = np.ascontiguousarray(cwt[:2].reshape(128, 512)).astype(bf)
    cwk2 = np.ascontiguousarray(cwt[2]).astype(bf)                 # [64, 512]
    iota = np.broadcast_to((np.arange(1024) % 64).astype(bf), (128, 1024))
    iota = np.ascontiguousarray(iota)
    i64d = np.concatenate([np.eye(64), np.eye(64)], axis=0).astype(bf)
    ieye = np.eye(128)
    id128 = np.concatenate([ieye[:, 0:64], ieye[:, 0:64],
                            ieye[:, 64:128], ieye[:, 64:128]], axis=1).astype(bf)
    consts = dict(w1t=w1t, w2t=w2t, cwk1=cwk1, cwk2=cwk2, iota=iota,
                  i64d=i64d, id128=id128)
    has_b1 = bool(np.any(b1))
    has_b2 = bool(np.any(b2))
    if has_b1:
        consts["b1c"] = np.ascontiguousarray(
            b1.reshape(4, 128).T).astype(np.float32)
    if has_b2:
        b2w = np.zeros((128, 98), np.float32)
        b2w[0:64, 1:97] = b2[None, :]
        b2w[0:64, 0] = b2[95]
        b2w[0:64, 97] = b2[0]
        b2w[64:128, 0:96] = b2[None, :]
        consts["b2w"] = b2w.astype(bf)
    return consts, has_b1, has_b2


_NC_CACHE = {}


def _get_nc(g_per_core, has_b1, has_b2):
    key = (g_per_core, has_b1, has_b2)
    if key in _NC_CACHE:
        return _NC_CACHE[key]
    npair = g_per_core // 2
    nc = bacc.Bacc("TRN2", target_bir_lowering=False, debug=False)
    ins = {
        "xt": nc.dram_tensor("xt", [128, npair * 96], BF16,
                             kind="ExternalInput").ap(),
        "et": nc.dram_tensor("et", [128, 512], BF16,
                             kind="ExternalInput").ap(),
        "w1t": nc.dram_tensor("w1t", [96, 512], BF16,
                              kind="ExternalInput").ap(),
        "w2t": nc.dram_tensor("w2t", [128, 384], BF16,
                              kind="ExternalInput").ap(),
        "cwk1": nc.dram_tensor("cwk1", [128, 512], BF16,
                               kind="ExternalInput").ap(),
        "cwk2": nc.dram_tensor("cwk2", [64, 512], BF16,
                               kind="ExternalInput").ap(),
        "iota": nc.dram_tensor("iota", [128, 1024], BF16,
                               kind="ExternalInput").ap(),
        "i64d": nc.dram_tensor("i64d", [128, 64], BF16,
                               kind="ExternalInput").ap(),
        "id128": nc.dram_tensor("id128", [128, 256], BF16,
                                kind="ExternalInput").ap(),
    }
    if has_b1:
        ins["b1c"] = nc.dram_tensor("b1c", [128, 4], FP32,
                                    kind="ExternalInput").ap()
    if has_b2:
        ins["b2w"] = nc.dram_tensor("b2w", [128, 98], BF16,
                                    kind="ExternalInput").ap()
    outs = {
        "y": nc.dram_tensor("y", [g_per_core, 96, 512], BF16,
                            kind="ExternalOutput").ap(),
    }
    with tile.TileContext(nc) as tc:
        build_gcn_kernel(tc, outs, ins, g_per_core, has_b1, has_b2)
    nc.compile()
    _NC_CACHE[key] = nc
    return nc


def kernel(x, edge_index, W1, b1, W2, b2, conv_w, _trace=False):
    bf = ml_dtypes.bfloat16
    x = np.asarray(x)
    edge_index = np.asarray(edge_index)
    consts, has_b1, has_b2 = _prep_consts(
        np.asarray(W1), np.asarray(b1), np.asarray(W2), np.asarray(b2),
        np.asarray(conv_w))
    nc = _get_nc(G, has_b1, has_b2)

    in_maps = []
    for c in range(N_CORES):
        sl = slice(c * G, (c + 1) * G)
        m = dict(consts)
        xc = x[sl]                                   # [G, 96, 64]
        xtc = (xc.transpose(0, 2, 1).reshape(NPAIR, 2, 64, 96)
               .transpose(1, 2, 0, 3).reshape(128, NPAIR * 96))
        m["xt"] = np.ascontiguousarray(xtc).astype(bf)
        eic = edge_index[sl].reshape(2 * G, 512)     # rows 2g+t
        etc = eic.reshape(128, 4, 128).transpose(2, 1, 0).reshape(128, 512)
        m["et"] = np.ascontiguousarray(etc).astype(bf)
        in_maps.append(m)

    res = run_bass_kernel_spmd(nc, in_maps, core_ids=list(range(N_CORES)),
                               trace=_trace)
    y = np.concatenate([res.results[c]["y"] for c in range(N_CORES)], axis=0)
    y = y.astype(np.float32)
    if _trace:
        kernel.last_results = res
    return y


# revision 59
# speedup vs baseline: 1.1125x; 1.1125x over previous
"""Trainium2 Bass kernel for batched GCN (2x GCNConv + circular Conv1d).

Math per graph (N=64 nodes, S=96 feats, H=512 hidden, E=512 edges):
    deg[d]   = #edges with dst=d (+1 self loop)
    Msb2     = Dinv A^T Dinv  (A[d,s] edge counts + I; Dinv = diag deg^-1/2)
    xa       = x @ Msb2                  # GCN1 aggregation first (reassoc)
    h1       = relu(W1 @ xa)             # [512, 64] (transposed layout)
    h2       = Msb2^T-agg(h1^T W2^T)     # [64, 96]
    y        = circular_conv1d(h2)       # [96, 512]

v2 design (cost-model driven):
  - Host pre-lays out x (node-major bf16) and edges (chunk-major bf16):
    no on-core transposes/casts for inputs.
  - One-hots via one DVE is_equal against an iota table with a broadcast
    input AP (no materialized broadcast copy).
  - A built block-diagonally in PSUM per pair (2 graphs); deg via 8
    free-size-1 matmuls; +1 self-loop via Sqrt bias.
  - Both dinv factors folded into the two PSUM->SBUF copies of A.
  - Conv with K-stacked taps: 2 matmuls per graph (taps{-1,0} K=128,
    tap{+1} K=64) over a halo'd shifted tile.
  - y stored bf16 (host upcasts), one DMA per pair, 1KB+ elems.
  - Elementwise spread across DVE/Act/Pool to sit under the PE bound.
"""

import numpy as np
import ml_dtypes

import concourse.bacc as bacc
import concourse.mybir as mybir
import concourse.tile as tile
from concourse.bass_utils import run_bass_kernel_spmd

BF16 = mybir.dt.bfloat16
FP32 = mybir.dt.float32
AF = mybir.ActivationFunctionType

N_CORES = 8
B, S, N, H, E = 512, 96, 64, 512, 512
G = B // N_CORES          # graphs per core
NPAIR = G // 2


def build_gcn_kernel(tc, outs, ins, g_per_core=G, has_b1=False, has_b2=False,
                     stage=6):
    nc = tc.nc
    g = g_per_core
    npair = g // 2

    xt_d = ins["xt"]        # [128, npair*96] bf16  (rows (gl,n), free (pr,s))
    et_d = ins["et"]        # [128, 512] bf16       (rows p, free (c, gt))
    w1t_d = ins["w1t"]      # [96, 512] bf16
    w2t_d = ins["w2t"]      # [128, 384] bf16  (rows h-in-chunk, free (c, s))
    cwk1_d = ins["cwk1"]    # [128, 512] bf16  (rows (k01, i), free o)
    cwk2_d = ins["cwk2"]    # [64, 512] bf16   (rows i (k=2), free o)
    iota_d = ins["iota"]    # [128, 1024] bf16 (f%64)
    i64d_d = ins["i64d"]    # [128, 64] bf16 (I64 stacked twice)
    id128_d = ins["id128"]  # [128, 128] bf16
    y_d = outs["y"]         # [g, 96, 512] bf16

    from contextlib import ExitStack
    ctx = ExitStack()
    const = ctx.enter_context(tc.tile_pool(name="const", bufs=1))
    sb = ctx.enter_context(tc.tile_pool(name="sb", bufs=4))
    sbh = ctx.enter_context(tc.tile_pool(name="sbh", bufs=3))
    pp = ctx.enter_context(tc.tile_pool(name="pp", bufs=2, space="PSUM"))
    pa = ctx.enter_context(tc.tile_pool(name="pa", bufs=2, space="PSUM"))
    pq = ctx.enter_context(tc.tile_pool(name="pq", bufs=2, space="PSUM"))
    py = ctx.enter_context(tc.tile_pool(name="py", bufs=2, space="PSUM"))

    # ---- constants ----
    xt = const.tile([128, npair * 96], BF16)
    nc.sync.dma_start(out=xt[:], in_=xt_d[:])
    et = const.tile([128, 512], BF16)
    nc.sync.dma_start(out=et[:], in_=et_d[:])
    w1t = const.tile([96, 512], BF16)
    nc.sync.dma_start(out=w1t[:], in_=w1t_d[:])
    w2t = const.tile([128, 384], BF16)
    nc.sync.dma_start(out=w2t[:], in_=w2t_d[:])
    cwk1 = const.tile([128, 512], BF16)
    nc.sync.dma_start(out=cwk1[:], in_=cwk1_d[:])
    cwk2 = const.tile([64, 512], BF16)
    nc.sync.dma_start(out=cwk2[:], in_=cwk2_d[:])
    iota = const.tile([128, 1024], BF16)
    nc.sync.dma_start(out=iota[:], in_=iota_d[:])
    i64d = const.tile([128, 64], BF16)
    nc.sync.dma_start(out=i64d[:], in_=i64d_d[:])
    # id_dup = [I[:, 0:64] | I[:, 0:64] | I[:, 64:128] | I[:, 64:128]] so the
    # transposing matmul emits each graph's Msb2 block twice, contiguously.
    iddup = const.tile([128, 256], BF16)
    nc.sync.dma_start(out=iddup[:], in_=id128_d[:])
    ones = const.tile([128, 1], BF16)
    nc.vector.memset(ones[:], 1.0)
    zeros64 = const.tile([128, 64], BF16)
    nc.vector.memset(zeros64[:], 0.0)
    if has_b1:
        b1c = const.tile([128, 4], FP32)
        nc.sync.dma_start(out=b1c[:], in_=ins["b1c"][:])
    if has_b2:
        b2w = const.tile([128, 98], BF16)
        nc.sync.dma_start(out=b2w[:], in_=ins["b2w"][:])

    et_c = et[:].rearrange("p (c gt) -> p c gt", c=4)

    for pr in range(npair):
        q = pr % 2          # position within quad
        if q == 0:
            a2q = pq.tile([128, 392], FP32, tag="a2q")

        # ---- one-hots: oh[p, (c, j, v)] = (et[p, (c, 4pr+j)] == v) ----
        e_sl = et_c[:, :, 4 * pr:4 * pr + 4]
        e_sl = e_sl.rearrange("p c (j u) -> p c j u", u=1)
        e_bc = e_sl.to_broadcast([128, 4, 4, 64])
        erep = sb.tile([128, 1024], BF16, tag="erep")
        nc.gpsimd.tensor_copy(
            out=erep[:].rearrange("p (c j v) -> p c j v", c=4, j=4),
            in_=e_bc)
        oh = sb.tile([128, 1024], BF16, tag="oh")
        nc.vector.tensor_tensor(
            out=oh[:], in0=erep[:], in1=iota[:],
            op=mybir.AluOpType.is_equal,
        )

        # ---- pair psum tile; mst reuses maug's columns (maug is dead
        # once msb has read it, and mst is written strictly after) ----
        mps = pp.tile([128, 496], FP32, tag="mps")
        maug = mps[:, 0:128]     # block-diag A+I, rows dst, cols src
        mst = mps[:, 0:256]      # [B0|B0|B1|B1] after dup-transpose
        deg = mps[:, 256:257]
        z2_ps = mps[:, 272:368]
        xa_ps = mps[0:96, 368:496]

        for gl in range(2):
            po = 64 * gl
            out_sl = maug[po:po + 64, po:po + 64]
            for c in range(4):
                base = c * 256
                lhsT = oh[:, base + (2 * gl + 1) * 64: base + (2 * gl + 2) * 64]
                rhs = oh[:, base + (2 * gl) * 64: base + (2 * gl + 1) * 64]
                nc.tensor.matmul(out_sl, lhsT, rhs, start=(c == 0), stop=False)
            nc.tensor.matmul(
                out_sl, i64d[po:po + 64, :], i64d[po:po + 64, :],
                start=False, stop=True)
            # zero the off-diag quadrant (rows gl, cols 1-gl)
            nc.tensor.matmul(
                maug[po:po + 64, 64 - po:128 - po],
                zeros64[:, :], oh[:, 0:64], start=True, stop=True)
            # deg[d] = edge count with dst=d (self-loop via Sqrt bias)
            dsl = deg[po:po + 64, :]
            for c in range(4):
                lhsT = oh[:, c * 256 + (2 * gl + 1) * 64:
                          c * 256 + (2 * gl + 2) * 64]
                nc.tensor.matmul(dsl, lhsT, ones[:], start=(c == 0),
                                 stop=(c == 3))

        # ---- dinv = 1/sqrt(deg+1) ----
        sq = sb.tile([128, 1], FP32, tag="sq")
        nc.scalar.activation(out=sq[:], in_=deg, func=AF.Sqrt, bias=1.0)
        dinv = sb.tile([128, 1], FP32, tag="dinv")
        nc.vector.reciprocal(out=dinv[:], in_=sq[:])

        # ---- msb = maug * dinv[dst] (block-diag, bf16) ----
        msb = sb.tile([128, 128], BF16, tag="msb")
        nc.vector.tensor_scalar(out=msb[:], in0=maug, scalar1=dinv[:, :],
                                scalar2=None, op0=mybir.AluOpType.mult)
        # ---- transpose-dup via matmul: mst = msb^T @ id_dup ----
        nc.tensor.matmul(mst, msb[:], iddup[:], start=True, stop=True)
        msbd = sb.tile([128, 256], BF16, tag="msbd")
        nc.scalar.activation(out=msbd[:], in_=mst, func=AF.Copy,
                             scale=dinv[:, :])

        if stage < 3:
            continue
        # ---- xa = x @ Msb2 : [96 (s), 128 (g,d)], one mm per col block ----
        nc.tensor.matmul(xa_ps[:, 0:64], xt[:, 96 * pr:96 * (pr + 1)],
                         msbd[:, 0:64], start=True, stop=True)
        nc.tensor.matmul(xa_ps[:, 64:128], xt[:, 96 * pr:96 * (pr + 1)],
                         msbd[:, 192:256], start=True, stop=True)
        xab = sb.tile([96, 128], BF16, tag="xab")
        nc.scalar.activation(out=xab[:], in_=xa_ps, func=AF.Copy)

        # ---- a1t = W1 @ xa : [128 (h-chunk), (c, g, d)] ----
        a1t = pa.tile([128, 512], FP32, tag="a1t")
        for c in range(4):
            nc.tensor.matmul(a1t[:, 128 * c:128 * (c + 1)],
                             w1t[:, 128 * c:128 * (c + 1)], xab[:],
                             start=True, stop=True)
        h1t = sb.tile([128, 512], BF16, tag="h1t")
        if has_b1:
            for c in range(4):
                nc.scalar.activation(
                    out=h1t[:, 128 * c:128 * (c + 1)],
                    in_=a1t[:, 128 * c:128 * (c + 1)],
                    func=AF.Relu, bias=b1c[:, c:c + 1])
        else:
            nc.scalar.activation(out=h1t[:], in_=a1t[:], func=AF.Relu)

        if stage < 4:
            continue
        # ---- z2 = h1 W2^T : [128 (g,n), 96 (s)] ----
        for c in range(4):
            nc.tensor.matmul(z2_ps, h1t[:, 128 * c:128 * (c + 1)],
                             w2t[:, 96 * c:96 * (c + 1)],
                             start=(c == 0), stop=(c == 3))
        z2s = sb.tile([128, 96], BF16, tag="z2s")
        nc.scalar.activation(out=z2s[:], in_=z2_ps, func=AF.Copy)

        # ---- a2 = Msb2^T-agg -> h2 duplicated into both row blocks ----
        # lhsT = [B_gl | B_gl] (contiguous in msbd thanks to the dup
        # transpose), so one M=128 matmul per graph writes rows 0:64 and
        # 64:128 identically. lhsT's off-graph rows are zero, so the full
        # 128-partition contraction selects graph gl alone (operands stay
        # at partition base 0). Circular-halo columns are written by two
        # extra free-size-1 matmuls, making a2q [halo|h2|halo] per graph.
        for gl in range(2):
            w0 = 98 * (2 * q + gl)
            lT = msbd[:, 128 * gl:128 * gl + 128]
            nc.tensor.matmul(a2q[0:128, w0 + 1:w0 + 97], lT, z2s[:, :],
                             start=True, stop=True)
            nc.tensor.matmul(a2q[0:128, w0:w0 + 1], lT, z2s[:, 95:96],
                             start=True, stop=True)
            nc.tensor.matmul(a2q[0:128, w0 + 97:w0 + 98], lT, z2s[:, 0:1],
                             start=True, stop=True)

        if q == 1 and stage < 5:
            for qq in range(4):
                ysb = sbh.tile([96, 512], BF16, tag="ysb")
                nc.vector.memset(ysb[:], 0.0)
                nc.sync.dma_start(out=y_d[4 * (pr // 2) + qq], in_=ysb[:])
        if q == 1 and stage >= 5:
            # ---- hpw [128, (4, 98)]: block0 = a2q verbatim (halo'd,
            # reads h2[l-1] at col l); block1 = h2[l] (shift via AP) ----
            hpw = sbh.tile([128, 392], BF16, tag="hpw")
            hpw1 = hpw[64:128, :].rearrange("p (qq w) -> p qq w", w=98)
            a2q1 = a2q[64:128, :].rearrange("p (qq w) -> p qq w", w=98)
            if has_b2:
                hpw0 = hpw[0:64, :].rearrange("p (qq w) -> p qq w", w=98)
                a2q0 = a2q[0:64, :].rearrange("p (qq w) -> p qq w", w=98)
                b2w0 = b2w[0:64, :].rearrange("p (qq w) -> p qq w", qq=1)
                b2w1 = b2w[64:128, :].rearrange("p (qq w) -> p qq w", qq=1)
                nc.vector.tensor_tensor(
                    out=hpw0[:, :, :], in0=a2q0[:, :, :],
                    in1=b2w0[:, :, :].to_broadcast([64, 4, 98]),
                    op=mybir.AluOpType.add)
                nc.vector.tensor_tensor(
                    out=hpw1[:, :, 0:96], in0=a2q1[:, :, 1:97],
                    in1=b2w1[:, :, 0:96].to_broadcast([64, 4, 96]),
                    op=mybir.AluOpType.add)
            else:
                nc.scalar.activation(out=hpw[0:64, :], in_=a2q[0:64, :],
                                     func=AF.Copy)
                nc.vector.tensor_copy(out=hpw1[:, :, 0:96],
                                      in_=a2q1[:, :, 1:97])

            # ---- conv: per graph 2 K-stacked matmuls -> [96 (l), 512 (o)] ----
            for qq in range(4):
                if qq % 2 == 0:
                    ysb = sbh.tile([96, 1024], BF16, tag="ysb")
                if stage < 6:
                    nc.vector.memset(ysb[:, 512 * (qq % 2):512 * (qq % 2 + 1)],
                                     0.0)
                else:
                    y_ps = py.tile([96, 512], FP32, tag="y")
                    nc.tensor.matmul(y_ps[:], hpw[:, 98 * qq:98 * qq + 96],
                                     cwk1[:], start=True, stop=False)
                    nc.tensor.matmul(y_ps[:],
                                     hpw[0:64, 98 * qq + 2:98 * qq + 98],
                                     cwk2[:], start=False, stop=True)
                    if qq % 2 == 0:
                        nc.vector.tensor_copy(out=ysb[:, 0:512], in_=y_ps[:])
                    else:
                        nc.scalar.activation(out=ysb[:, 512:1024],
                                             in_=y_ps[:], func=AF.Copy)
                if qq % 2 == 1:
                    gb = 4 * (pr // 2) + qq - 1
                    nc.sync.dma_start(
                        out=y_d[gb:gb + 2].rearrange("g l o -> l g o"),
                        in_=ysb[:].rearrange("l (g o) -> l g o", g=2))

    ctx.close()


# ---------------- host side ----------------

def _prep_consts(W1, b1, W2, b2, conv_w):
    bf = ml_dtypes.bfloat16
    w1t = np.ascontiguousarray(W1.T).astype(bf)                    # [96, 512]
    w2t = np.ascontiguousarray(
        W2.T.reshape(4, 128, 96).transpose(1, 0, 2).reshape(128, 384)
    ).astype(bf)
    cwt = conv_w.transpose(2, 1, 0)                                # [3, 64, 512]
    cwk1 = np.ascontiguousarray(cwt[:2].reshape(128, 512)).astype(bf)
    cwk2 = np.ascontiguousarray(cwt[2]).astype(bf)                 # [64, 512]
    iota = np.broadcast_to((np.arange(1024) % 64).astype(bf), (128, 1024))
    iota = np.ascontiguousarray(iota)
    i64d = np.concatenate([np.eye(64), np.eye(64)], axis=0).astype(bf)
    ieye = np.eye(128)
    id128 = np.concatenate([ieye[:, 0:64], ieye[:, 0:64],
                            ieye[:, 64:128], ieye[:, 64:128]], axis=1).astype(bf)
    consts = dict(w1t=w1t, w2t=w2t, cwk1=cwk1, cwk2=cwk2, iota=iota,
                  i64d=i64d, id128=id128)
    has_b1 = bool(np.any(b1))
    has_b2 = bool(np.any(b2))
    if has_b1:
        consts["b1c"] = np.ascontiguousarray(
            b1.reshape(4, 128).T).astype(np.float32)
    if has_b2:
        b2w = np.zeros((128, 98), np.float32)
        b2w[0:64, 1:97] = b2[None, :]
        b2w[0:64, 0] = b2[95]
        b2w[0:64, 97] = b2[0]
        b2w[64:128, 0:96] = b2[None, :]
        consts["b2w"] = b2w.astype(bf)
    return consts, has_b1, has_b2


_NC_CACHE = {}


def _get_nc(g_per_core, has_b1, has_b2):
    key = (g_per_core, has_b1, has_b2)
    if key in _NC_CACHE:
        return _NC_CACHE[key]
    npair = g_per_core // 2
    nc = bacc.Bacc("TRN2", target_bir_lowering=False, debug=False)
    ins = {
        "xt": nc.dram_tensor("xt", [128, npair * 96], BF16,
                             kind="ExternalInput").ap(),
        "et": nc.dram_tensor("et", [128, 512], BF16,
                             kind="ExternalInput").ap(),
        "w1t": nc.dram_tensor("w1t", [96, 512], BF16,
                              kind="ExternalInput").ap(),
        "w2t": nc.dram_tensor("w2t", [128, 384], BF16,
                              kind="ExternalInput").ap(),
        "cwk1": nc.dram_tensor("cwk1", [128, 512], BF16,
                               kind="ExternalInput").ap(),
        "cwk2": nc.dram_tensor("cwk2", [64, 512], BF16,
                               kind="ExternalInput").ap(),
        "iota": nc.dram_tensor("iota", [128, 1024], BF16,
                               kind="ExternalInput").ap(),
        "i64d": nc.dram_tensor("i64d", [128, 64], BF16,
                               kind="ExternalInput").ap(),
        "id128": nc.dram_tensor("id128", [128, 256], BF16,
                                kind="ExternalInput").ap(),
    }
    if has_b1:
        ins["b1c"] = nc.dram_tensor("b1c", [128, 4], FP32,
                                    kind="ExternalInput").ap()
    if has_b2:
        ins["b2w"] = nc.dram_tensor("b2w", [128, 98], BF16,
                                    kind="ExternalInput").ap()
    outs = {
        "y": nc.dram_tensor("y", [g_per_core, 96, 512], BF16,
                            kind="ExternalOutput").ap(),
    }
    with tile.TileContext(nc) as tc:
        build_gcn_kernel(tc, outs, ins, g_per_core, has_b1, has_b2)
    nc.compile()
    _NC_CACHE[key] = nc
    return nc


def kernel(x, edge_index, W1, b1, W2, b2, conv_w, _trace=False):
    bf = ml_dtypes.bfloat16
    x = np.asarray(x)
    edge_index = np.asarray(edge_index)
    consts, has_b1, has_b2 = _prep_consts(
        np.asarray(W1), np.asarray(b1), np.asarray(W2), np.asarray(b2),
        np.asarray(conv_w))
    nc = _get_nc(G, has_b1, has_b2)

    in_maps = []
    for c in range(N_CORES):
        sl = slice(c * G, (c + 1) * G)
        m = dict(consts)
        xc = x[sl]                                   # [G, 96, 64]
        xtc = (xc.transpose(0, 2, 1).reshape(NPAIR, 2, 64, 96)
               .transpose(1, 2, 0, 3).reshape(128, NPAIR * 96))
        m["xt"] = np.ascontiguousarray(xtc).astype(bf)
        eic = edge_index[sl].reshape(2 * G, 512)     # rows 2g+t
        etc = eic.reshape(128, 4, 128).transpose(2, 1, 0).reshape(128, 512)
        m["et"] = np.ascontiguousarray(etc).astype(bf)
        in_maps.append(m)

    res = run_bass_kernel_spmd(nc, in_maps, core_ids=list(range(N_CORES)),
                               trace=_trace)
    y = np.concatenate([res.results[c]["y"] for c in range(N_CORES)], axis=0)
    y = y.astype(np.float32)
    if _trace:
        kernel.last_results = res
    return y
